# revision 1
# baseline (speedup 1.0000x reference)
"""GPT2 full (non-causal) attention on 8 trn2 NeuronCores.

Sharding: core c -> (batch b = c//2, head-group g = c%2 of 8 heads).
Each core computes its batch rows x its 8 heads end-to-end:
  qkv^T projection -> scores S^T = K Q^T (row-packed head pairs on PE)
  -> exp (ScalarE, no max-subtraction: |scores/8| <~ 3 with this data
  distribution so exp is stable) -> AV matmul with an appended
  ones-column on V to produce softmax denominators -> normalize
  -> partial out-projection over this core's 512 C_in channels.
Host sums the two partial projections per batch and adds b_proj.

All matmuls in bf16 (PE runs fp32 matmul at 1/4 rate), fp32 PSUM
accumulation. Denominator/numerator share the same bf16 P so the
softmax ratio rounding largely cancels.
"""

import os
import sys

sys.path.insert(0, "/opt/trn_rl_repo")

import numpy as np
import ml_dtypes

import concourse.bass as bass
import concourse.tile as tile
from concourse import bacc, mybir
from concourse.bass import ts, ds

B, T, C, H, D = 4, 2048, 1024, 16, 64
NCORES = 8
HPC = H // 2          # heads per core = 8
CIN = HPC * D         # per-core proj contraction channels = 512
NQ = 512              # query chunk (matmul moving free dim)
F32 = mybir.dt.float32
BF16 = mybir.dt.bfloat16
BF = ml_dtypes.bfloat16

_cache = {}


def build_program():
    nc = bacc.Bacc("TRN2", target_bir_lowering=False, debug=False)
    xT = nc.dram_tensor("xT", [C, T], BF16, kind="ExternalInput").ap()
    wq = nc.dram_tensor("wq", [C, CIN], BF16, kind="ExternalInput").ap()
    wk = nc.dram_tensor("wk", [C, CIN], BF16, kind="ExternalInput").ap()
    wv = nc.dram_tensor("wv", [C, CIN], BF16, kind="ExternalInput").ap()
    bqk = nc.dram_tensor("bqk", [2 * CIN, 1], F32, kind="ExternalInput").ap()
    bv = nc.dram_tensor("bv", [1, CIN], F32, kind="ExternalInput").ap()
    wp = nc.dram_tensor("wp", [CIN, C], BF16, kind="ExternalInput").ap()
    out = nc.dram_tensor("out", [T, C], F32, kind="ExternalOutput").ap()

    with tile.TileContext(nc) as tc:
        _build_kernel(tc, xT, wq, wk, wv, bqk, bv, wp, out)
    nc.compile()
    return nc


def _build_kernel(tc, xT, wq, wk, wv, bqk, bv, wp, out):
    nc = tc.nc
    mm = nc.tensor.matmul
    KT = C // 128        # 8 contraction tiles for qkv projection
    RT = T // 128        # 16 row/key tiles
    NCH = T // NQ        # 4 query chunks
    EXP = mybir.ActivationFunctionType.Exp
    import contextlib

    with contextlib.ExitStack() as ctx:
        consts = ctx.enter_context(tc.tile_pool(name="consts", bufs=1))
        wpool = ctx.enter_context(tc.tile_pool(name="wpool", bufs=1))
        xpool = ctx.enter_context(tc.tile_pool(name="xpool", bufs=1))
        qkpool = ctx.enter_context(tc.tile_pool(name="qkpool", bufs=1))
        vpool = ctx.enter_context(tc.tile_pool(name="vpool", bufs=1))
        ppool = ctx.enter_context(tc.tile_pool(name="ppool", bufs=20))
        ypool = ctx.enter_context(tc.tile_pool(name="ypool", bufs=8))
        opool = ctx.enter_context(tc.tile_pool(name="opool", bufs=4))
        small = ctx.enter_context(tc.tile_pool(name="small", bufs=2))
        s_ps = ctx.enter_context(tc.tile_pool(name="s_ps", bufs=2, space="PSUM"))
        y_ps = ctx.enter_context(tc.tile_pool(name="y_ps", bufs=1, space="PSUM"))
        p_ps = ctx.enter_context(tc.tile_pool(name="p_ps", bufs=2, space="PSUM"))

        # ---- load inputs -------------------------------------------------
        xt_sb = []
        for i in range(KT):
            xts = xpool.tile([128, T], BF16, name=f"xts{i}")
            nc.sync.dma_start(xts[:], xT[ts(i, 128), :])
            xt_sb.append(xts)
        wq_sb, wk_sb, wv_sb = [], [], []
        for w_dram, lst, nm in ((wq, wq_sb, "wq"), (wk, wk_sb, "wk"), (wv, wv_sb, "wv")):
            for i in range(KT):
                wt = wpool.tile([128, CIN], BF16, name=f"{nm}s{i}")
                nc.sync.dma_start(wt[:], w_dram[ts(i, 128), :])
                lst.append(wt)
        wp_sb = []
        for i in range(CIN // 128):
            wpt = wpool.tile([128, C], BF16, name=f"wps{i}")
            nc.sync.dma_start(wpt[:], wp[ts(i, 128), :])
            wp_sb.append(wpt)
        bqk_sb = []
        for m in range(8):
            bt = consts.tile([128, 1], F32, name=f"bqk{m}")
            nc.sync.dma_start(bt[:], bqk[ts(m, 128), :])
            bqk_sb.append(bt)
        bv_sb = consts.tile([1, CIN], F32, name="bv_sb")
        nc.sync.dma_start(bv_sb[:], bv[:])
        bvb = consts.tile([128, CIN], F32, name="bvb")
        nc.gpsimd.partition_broadcast(bvb[:], bv_sb[:], channels=128)

        # ---- qkv projection ---------------------------------------------
        # qkT[m] (m 0-3: q^T cols, 4-7: k^T cols): [128 cols, T]
        qkT = [qkpool.tile([128, T], BF16, name=f"qkT{m}") for m in range(8)]
        for m in range(8):
            w_sb = wq_sb if m < 4 else wk_sb
            mc = (m % 4) * 128
            for n2 in range(NCH // 2):
                sp = s_ps.tile([128, 1024], F32, tag="sp")
                for half in range(2):
                    n = 2 * n2 + half
                    for kc in range(KT):
                        mm(sp[:, ts(half, 512)], w_sb[kc][:, ds(mc, 128)],
                           xt_sb[kc][:, ts(n, NQ)],
                           start=(kc == 0), stop=(kc == KT - 1))
                nc.vector.tensor_scalar_add(qkT[m][:, ts(n2, 1024)], sp[:], bqk_sb[m][:])

        # v natural layout with ones column: v_sb[r][p, h, 0:64] = v rows,
        # v_sb[r][p, h, 64] = 1.0 (softmax denominator trick)
        v_sb = [vpool.tile([128, HPC, D + 1], BF16, name=f"vsb{r}") for r in range(RT)]
        for r2 in range(RT // 2):
            sp = s_ps.tile([128, 1024], F32, tag="sp")
            for half in range(2):
                r = 2 * r2 + half
                for kc in range(KT):
                    mm(sp[:, ts(half, 512)], xt_sb[kc][:, ts(r, 128)], wv_sb[kc][:],
                       start=(kc == 0), stop=(kc == KT - 1))
            for half in range(2):
                r = 2 * r2 + half
                nc.vector.tensor_add(
                    v_sb[r][:, :, 0:D],
                    sp[:, ts(half, 512)].rearrange("p (h d) -> p h d", h=HPC),
                    bvb[:].rearrange("p (h d) -> p h d", h=HPC))
                nc.vector.memset(v_sb[r][:, :, D:D + 1], 1.0)

        # ---- attention + out projection, streamed per query chunk --------
        for qc in range(NCH):
            yp_tiles = []
            for hp in range(4):
                kt_t = qkT[4 + hp]
                qt_t = qkT[hp]
                p_tiles = []
                for kt in range(RT):
                    sp = s_ps.tile([128, 1024], F32, tag="sp")
                    mm(sp[:, 0:512], kt_t[0:64, ts(kt, 128)], qt_t[0:64, ts(qc, NQ)],
                       start=True, stop=True)
                    mm(sp[:, 512:1024], kt_t[64:128, ts(kt, 128)], qt_t[64:128, ts(qc, NQ)],
                       start=True, stop=True, tile_position=(64, 0))
                    pp = ppool.tile([128, 1024], BF16, tag="pp")
                    nc.scalar.activation(pp[:], sp[:], EXP, scale=0.125)
                    p_tiles.append(pp)
                ya = y_ps.tile([D + 1, NQ], F32, tag="ya")
                yb = y_ps.tile([D + 1, NQ], F32, tag="yb")
                for kt in range(RT):
                    mm(ya[:], v_sb[kt][:, 2 * hp, :], p_tiles[kt][:, 0:512],
                       start=(kt == 0), stop=(kt == RT - 1))
                for kt in range(RT):
                    mm(yb[:], v_sb[kt][:, 2 * hp + 1, :], p_tiles[kt][:, 512:1024],
                       start=(kt == 0), stop=(kt == RT - 1))
                reca = small.tile([1, NQ], F32, tag="reca")
                recb = small.tile([1, NQ], F32, tag="recb")
                nc.vector.reciprocal(reca[:], ya[D:D + 1, :])
                nc.vector.reciprocal(recb[:], yb[D:D + 1, :])
                bca = small.tile([D, NQ], F32, tag="bca")
                bcb = small.tile([D, NQ], F32, tag="bcb")
                nc.gpsimd.partition_broadcast(bca[:], reca[:], channels=D)
                nc.gpsimd.partition_broadcast(bcb[:], recb[:], channels=D)
                yp = ypool.tile([128, NQ], BF16, tag="yp")
                nc.vector.tensor_mul(yp[0:D, :], ya[0:D, :], bca[:])
                nc.vector.tensor_mul(yp[D:2 * D, :], yb[0:D, :], bcb[:])
                yp_tiles.append(yp)
            for mt in range(NQ // 128):
                for n2 in range(C // 512):
                    prp = p_ps.tile([128, 512], F32, tag="prp")
                    for hp in range(4):
                        mm(prp[:], yp_tiles[hp][:, ts(mt, 128)], wp_sb[hp][:, ts(n2, 512)],
                           start=(hp == 0), stop=(hp == 3))
                    ot = opool.tile([128, 512], F32, tag="ot")
                    nc.vector.tensor_copy(ot[:], prp[:])
                    nc.sync.dma_start(out[ds(qc * NQ + mt * 128, 128), ts(n2, 512)], ot[:])


def _prep_core_inputs(x, W_attn, b_attn, W_proj):
    """Slice + cast host-side into per-core input maps."""
    in_maps = []
    for c in range(NCORES):
        b, g = c // 2, c % 2
        xT_c = np.ascontiguousarray(x[b].T).astype(BF)
        qs, ks, vs = g * CIN, C + g * CIN, 2 * C + g * CIN
        in_maps.append({
            "xT": xT_c,
            "wq": np.ascontiguousarray(W_attn[:, qs:qs + CIN]).astype(BF),
            "wk": np.ascontiguousarray(W_attn[:, ks:ks + CIN]).astype(BF),
            "wv": np.ascontiguousarray(W_attn[:, vs:vs + CIN]).astype(BF),
            "bqk": np.concatenate(
                [b_attn[qs:qs + CIN], b_attn[ks:ks + CIN]]
            ).reshape(2 * CIN, 1).astype(np.float32),
            "bv": b_attn[vs:vs + CIN].reshape(1, CIN).astype(np.float32),
            "wp": np.ascontiguousarray(W_proj[g * CIN:(g + 1) * CIN, :]).astype(BF),
        })
    return in_maps


def kernel(x, W_attn, b_attn, W_proj, b_proj, _trace=False):
    x = np.asarray(x, dtype=np.float32)
    W_attn = np.asarray(W_attn, dtype=np.float32)
    b_attn = np.asarray(b_attn, dtype=np.float32)
    W_proj = np.asarray(W_proj, dtype=np.float32)
    b_proj = np.asarray(b_proj, dtype=np.float32)

    if "nc" not in _cache:
        _cache["nc"] = build_program()
    nc = _cache["nc"]

    from concourse import bass_utils
    in_maps = _prep_core_inputs(x, W_attn, b_attn, W_proj)
    res = bass_utils.run_bass_kernel_spmd(
        nc, in_maps, core_ids=list(range(NCORES)), trace=_trace)
    _cache["last_result"] = res

    outs = [r["out"].astype(np.float32) for r in res.results]
    y = np.empty((B, T, C), dtype=np.float32)
    for b in range(B):
        y[b] = outs[2 * b] + outs[2 * b + 1] + b_proj[None, :]
    return y
